# revision 52
# baseline (speedup 1.0000x reference)
"""Trainium2 Bass kernel for nn_BaseAttention (sliding-window attention).

Full-input contract: kernel(x, Wqkv) -> [B, T, C] float32.

Sharding (8 cores): data-parallel over B (2) x tensor-parallel over head
groups (16 heads -> 4 groups of 4). Core c handles batch c//4, head group
c%4. Each core computes its QKV projection slice (768 of 3072 output rows)
and banded attention for its 4 heads; outputs are disjoint channel slices
of the final [B, T, C] tensor, so no collectives are needed.

Device-side design (per core), v2 -- engine-balanced and fully pipelined:

  * Projection (fp32r, all matmul free dims >= 256 so fp32r runs at full
    PE rate): q/k land transposed [d_part, T]; v lands [t_part, d] and is
    stored bf16 with a ones column appended ([v | 1]).
  * Attention runs KEY-chunk-major: for key chunk jb, ONE score matmul per
    head produces scores^T [128 keys, 384 queries] (query blocks jb-1..jb+1)
    -- free dim 384 >= 256 keeps fp32r at 1 cycle/row (the old per-block
    [128,128] scores ran at 1/4 rate).
  * exp on the scalar engine over two heads at once (strided AP) amortizes
    the ~220ns fixed access cost; sliding-window masking is two 0/1
    multiplies (all-4-heads wide, bf16/SBUF fast path) on the vector engine
    covering the two edge thirds only.
  * PV is FLIPPED: matmul(out[q,65], lhsT=p^T chunk, rhs=[v|1]) accumulates
    the output block in [query, d] orientation directly, so the old
    PSUM-copy + PE-transpose per (block, head) disappears entirely. Column
    64 is the softmax denominator l; normalize-and-evict is a per-partition
    tensor_scalar multiply with 1/l.
  * Emission interleaves projection slices with attention chunk groups
    (chunk jb only needs t < (jb+2)*128), keeping the PE continuously busy
    (it ramps 0.65->2.4 GHz only after ~3us of uninterrupted work) and
    overlapping attention's scalar/vector load with projection's PE load.
  * Engine split: exp->scalar, masks+recip+qk-evict->vector, v-evict->
    scalar, normalize-evict->gpsimd, DMA issue split sync/gpsimd.
"""

import os
import sys

import numpy as np

if "/opt/trn_rl_repo" not in sys.path:
    sys.path.insert(0, "/opt/trn_rl_repo")

B, T, C = 2, 2048, 1024
HEADS = 16
D = C // HEADS  # 64
WINDOW = 128
N_CORES = 8
HPC = HEADS // 4  # heads per core (4)
OPC = 3 * HPC * D  # projection output rows per core (768)

PDT_NAME = os.environ.get("SA_PDT", "bf16")

_PROGRAM_CACHE = {}


def _build_program(pdt_name):
    import concourse.mybir as mybir
    from concourse import bacc
    import concourse.tile as tile
    from contextlib import ExitStack

    f32 = mybir.dt.float32
    f32r = mybir.dt.float32r
    bf16 = mybir.dt.bfloat16
    PDT = bf16 if pdt_name == "bf16" else f32r
    Exp = mybir.ActivationFunctionType.Exp

    nc = bacc.Bacc()
    xT_d = nc.declare_dram_parameter("xT", [C, T], bf16, isOutput=False)
    wT_d = nc.declare_dram_parameter("wT", [C, OPC], bf16, isOutput=False)
    msk_d = nc.declare_dram_parameter("msk", [128, 2, HPC, 128], bf16, isOutput=False)
    out_d = nc.declare_dram_parameter("out", [T, HPC * D], f32, isOutput=True)

    CC = C // 128  # 8 contraction chunks
    TS = 512  # projection t-slice
    NS = T // TS  # 4 slices
    NB = T // 128  # 16 key chunks / query blocks

    with ExitStack() as ctx:
        tc = ctx.enter_context(tile.TileContext(nc))
        const = ctx.enter_context(tc.tile_pool(name="const", bufs=1))
        xpool = ctx.enter_context(tc.tile_pool(name="xp", bufs=4))
        ppool = ctx.enter_context(tc.tile_pool(name="pp", bufs=5))
        rpool = ctx.enter_context(tc.tile_pool(name="rp", bufs=2))
        qk_ps = ctx.enter_context(tc.tile_pool(name="qkps", bufs=2, space="PSUM"))
        ov_ps = ctx.enter_context(tc.tile_pool(name="ovps", bufs=2, space="PSUM"))
        ps_const = ctx.enter_context(tc.tile_pool(name="psc", bufs=1, space="PSUM"))

        # ---- constants / persistent tiles ----
        w_sb = const.tile([128, CC, OPC], bf16)
        mskb = const.tile([128, 2, HPC, 128], PDT)
        q_sb = const.tile([128, 2, T], bf16)
        k_sb = const.tile([128, 2, T], bf16)
        v_sb = const.tile([128, NB, HPC, D + 1], PDT)
        o_sb = const.tile([128, NB, HPC * D], f32)
        wT_r = wT_d.rearrange("(cc p) o -> p cc o", p=128)

        # PSUM: qk 2 banks, sct4 4 banks, ov 2 banks = 8 total.
        # A matmul with start=True zeroes its whole 2KB PSUM bank ("zero
        # region"), so concurrent accumulators must each own a bank: sct4
        # pads each head's 384-col scores to a 512-col bank, and each ov
        # accumulation is a CONTIGUOUS 3-matmul chain into a bank-sized
        # rotating buffer (all three p^T chunk tiles are alive at once).
        sct4 = ps_const.tile([128, HPC, 512], f32)

        xT_r = xT_d.rearrange("(cc p) t -> p cc t", p=128)
        xs_tiles = {}

        def emit_dma_x(s):
            xs = xpool.tile([128, CC, TS], bf16, tag="xs", name=f"xs{s}")
            xs_tiles[s] = xs
            for c in range(CC):
                eng = nc.sync if c % 2 == 0 else nc.gpsimd
                eng.dma_start(out=xs[:, c, :], in_=xT_r[:, c, s * TS:(s + 1) * TS])

        # x slice 0 and weights fan out over FOUR issue queues (vector and
        # scalar are idle until attention starts), x0[c] and w[c] on
        # different queues so the first chain's pairs land in parallel.
        xs0 = xpool.tile([128, CC, TS], bf16, tag="xs", name="xs0")
        xs_tiles[0] = xs0
        for c in range(CC):
            eng_a = nc.sync if c % 2 == 0 else nc.gpsimd
            eng_b = nc.gpsimd if c % 2 == 0 else nc.sync
            eng_a.dma_start(out=xs0[:, c, :], in_=xT_r[:, c, 0:TS])
            eng_b.dma_start(out=w_sb[:, c, :], in_=wT_r[:, c, :])
        nc.sync.dma_start(out=mskb, in_=msk_d[:, :, :, :])
        nc.vector.memset(v_sb[:, :, :, D:D + 1], 1.0)

        def emit_proj_qk(s):
            xs = xs_tiles[s]
            for m in range(4):
                ps = qk_ps.tile([128, TS], f32, tag="ps", name=f"ps{s}_{m}")
                for c in range(CC):
                    nc.tensor.matmul(
                        ps,
                        lhsT=w_sb[:, c, m * 128:(m + 1) * 128],
                        rhs=xs[:, c, :],
                        start=(c == 0),
                        stop=(c == CC - 1),
                    )
                dst = (q_sb if m < 2 else k_sb)[:, m % 2, s * TS:(s + 1) * TS]
                nc.scalar.copy(dst, ps)

        def emit_proj_v(s, t4):
            xs = xs_tiles[s]
            pv = qk_ps.tile([128, TS], f32, tag="ps", name=f"pv{s}_{t4}")
            pv = pv[:, 0:HPC * D]
            for c in range(CC):
                nc.tensor.matmul(
                    pv,
                    lhsT=xs[:, c, t4 * 128:(t4 + 1) * 128],
                    rhs=w_sb[:, c, 2 * HPC * D:3 * HPC * D],
                    start=(c == 0),
                    stop=(c == CC - 1),
                )
            tb = s * 4 + t4
            nc.scalar.copy(
                v_sb[:, tb, :, 0:D], pv.rearrange("p (h d) -> p h d", h=HPC)
            )

        p_tiles = {}

        def emit_sc(jb):
            lo, hi = max(jb - 1, 0), min(jb + 1, NB - 1)
            width = (hi - lo + 1) * 128
            for h in range(HPC):
                mt, po = divmod(h, 2)
                po *= 64
                nc.tensor.matmul(
                    sct4[:, h, 0:width],
                    lhsT=k_sb[po:po + 64, mt, jb * 128:(jb + 1) * 128],
                    rhs=q_sb[po:po + 64, mt, lo * 128:(hi + 1) * 128],
                    start=True,
                    stop=True,
                )
            p_t = ppool.tile([128, HPC, 384], PDT, tag="p", name=f"p{jb}")
            p_tiles[jb] = p_t
            if jb >= 12:
                # post-projection chunks: the scalar queue is exp-only here,
                # so pair-split releases the sct-bank WAR half a chunk sooner.
                for pr in range(2):
                    hh = slice(2 * pr, 2 * pr + 2)
                    nc.scalar.activation(
                        p_t[:, hh, 0:width], sct4[:, hh, 0:width], Exp
                    )
            else:
                nc.scalar.activation(p_t[:, :, 0:width], sct4[:, :, 0:width], Exp)
            # edge thirds: seg of q=jb-1 needs (c >= r), seg of q=jb+1 (c <= r)
            off = (hi - lo) * 128
            if lo == jb - 1:
                nc.vector.tensor_mul(
                    p_t[:, :, 0:128], p_t[:, :, 0:128], mskb[:, 1, :, :]
                )
            if hi == jb + 1:
                nc.vector.tensor_mul(
                    p_t[:, :, off:off + 128], p_t[:, :, off:off + 128],
                    mskb[:, 0, :, :],
                )

        def emit_pv(q):
            # one contiguous accumulation chain per (q, h); the chain reads
            # p^T segments out of the (up to) three live p_t tiles.
            jbs = [jb for jb in (q - 1, q, q + 1) if 0 <= jb < NB]
            for h in range(HPC):
                ov = ov_ps.tile([128, 512], f32, tag="ov", name=f"ov{q}_{h}")
                for n, jb in enumerate(jbs):
                    seg = q - max(jb - 1, 0)
                    nc.tensor.matmul(
                        ov[:, 0:D + 1],
                        lhsT=p_tiles[jb][:, h, seg * 128:(seg + 1) * 128],
                        rhs=v_sb[:, jb, h, :],
                        start=(n == 0),
                        stop=(n == len(jbs) - 1),
                    )
                r_t = rpool.tile([128, 1], f32, tag=f"r{h}", name=f"r{q}_{h}")
                nc.vector.reciprocal(r_t, ov[:, D:D + 1])
                nc.vector.tensor_scalar_mul(
                    o_sb[:, q, h * D:(h + 1) * D], ov[:, 0:D], r_t
                )
            out_r = out_d.rearrange("(q p) o -> p q o", p=128)
            if q >= 12:
                # singles at the end so the final transfer is short
                nc.sync.dma_start(out=out_r[:, q:q + 1, :], in_=o_sb[:, q:q + 1, :])
            elif q % 4 == 3:
                nc.sync.dma_start(
                    out=out_r[:, q - 3:q + 1, :], in_=o_sb[:, q - 3:q + 1, :]
                )

        # ---- interleaved emission ----
        # chunk jb needs q/k of slice (jb+2)//4 and v block jb+1, so one new
        # chunk becomes runnable after each v t-block: jb = 4*s + t4 - 1.
        # PV chains lag score emission by 2 chunks so exp(jb) has a full
        # chunk of slack before the PE needs p_t(jb).
        emit_dma_x(1)
        emit_dma_x(2)
        for s in range(NS):
            emit_proj_qk(s)
            for t4 in range(4):
                emit_proj_v(s, t4)
                jb = 4 * s + t4 - 1
                if jb >= 0:
                    emit_sc(jb)
                    if jb >= 2:
                        emit_pv(jb - 2)
                        p_tiles.pop(jb - 3, None)
            if s == 0:
                emit_dma_x(3)
        emit_sc(NB - 1)
        for q in (NB - 3, NB - 2, NB - 1):
            emit_pv(q)

    nc.compile()
    return nc


def _host_inputs(x, Wqkv):
    """Per-core input maps: shard batch x head-group, pre-transpose, bf16."""
    import ml_dtypes

    bf16 = ml_dtypes.bfloat16
    scale = float(D) ** -0.5
    r = np.arange(128, dtype=np.float32)[:, None]
    ci = np.arange(128, dtype=np.float32)[None, :]
    # type 0: (c <= r)  applied to the seg of query block jb+1
    # type 1: (c >= r)  applied to the seg of query block jb-1
    msk = np.stack([(ci <= r), (ci >= r)], axis=1).astype(np.float32)  # [128,2,128]
    msk = np.repeat(msk[:, :, None, :], HPC, axis=2)  # [128, 2, HPC, 128]
    msk = np.ascontiguousarray(msk).astype(bf16)

    x = np.asarray(x, dtype=np.float32)
    Wqkv = np.asarray(Wqkv, dtype=np.float32)
    xT = [np.ascontiguousarray(x[b].T).astype(bf16) for b in range(B)]
    in_maps = []
    for core in range(N_CORES):
        b, hg = divmod(core, N_CORES // B)
        rows = slice(hg * HPC * D, (hg + 1) * HPC * D)
        wcat = np.concatenate(
            [
                Wqkv[0 * C:1 * C][rows] * scale,
                Wqkv[1 * C:2 * C][rows],
                Wqkv[2 * C:3 * C][rows],
            ],
            axis=0,
        )
        in_maps.append(
            {
                "xT": xT[b],
                "wT": np.ascontiguousarray(wcat.T).astype(bf16),
                "msk": msk,
            }
        )
    return in_maps


def _gather(results):
    out = np.empty((B, T, C), dtype=np.float32)
    for core in range(N_CORES):
        b, hg = divmod(core, N_CORES // B)
        out[b, :, hg * HPC * D:(hg + 1) * HPC * D] = results[core]["out"]
    return out


def kernel(x, Wqkv):
    from concourse.bass_utils import run_bass_kernel_spmd

    key = PDT_NAME
    if key not in _PROGRAM_CACHE:
        _PROGRAM_CACHE[key] = _build_program(key)
    nc = _PROGRAM_CACHE[key]
    in_maps = _host_inputs(x, Wqkv)
    res = run_bass_kernel_spmd(nc, in_maps, list(range(N_CORES)))
    return _gather(res.results)


# revision 53
# speedup vs baseline: 1.1429x; 1.1429x over previous
"""Trainium2 Bass kernel for nn_BaseAttention (sliding-window attention).

Full-input contract: kernel(x, Wqkv) -> [B, T, C] float32.

Sharding (8 cores): data-parallel over B (2) x tensor-parallel over head
groups (16 heads -> 4 groups of 4). Core c handles batch c//4, head group
c%4. Each core computes its QKV projection slice (768 of 3072 output rows)
and banded attention for its 4 heads; outputs are disjoint channel slices
of the final [B, T, C] tensor, so no collectives are needed.

Device-side design (per core), v2 -- engine-balanced and fully pipelined:

  * Projection (fp32r, all matmul free dims >= 256 so fp32r runs at full
    PE rate): q/k land transposed [d_part, T]; v lands [t_part, d] and is
    stored bf16 with a ones column appended ([v | 1]).
  * Attention runs KEY-chunk-major: for key chunk jb, ONE score matmul per
    head produces scores^T [128 keys, 384 queries] (query blocks jb-1..jb+1)
    -- free dim 384 >= 256 keeps fp32r at 1 cycle/row (the old per-block
    [128,128] scores ran at 1/4 rate).
  * exp on the scalar engine over two heads at once (strided AP) amortizes
    the ~220ns fixed access cost; sliding-window masking is two 0/1
    multiplies (all-4-heads wide, bf16/SBUF fast path) on the vector engine
    covering the two edge thirds only.
  * PV is FLIPPED: matmul(out[q,65], lhsT=p^T chunk, rhs=[v|1]) accumulates
    the output block in [query, d] orientation directly, so the old
    PSUM-copy + PE-transpose per (block, head) disappears entirely. Column
    64 is the softmax denominator l; normalize-and-evict is a per-partition
    tensor_scalar multiply with 1/l.
  * Emission interleaves projection slices with attention chunk groups
    (chunk jb only needs t < (jb+2)*128), keeping the PE continuously busy
    (it ramps 0.65->2.4 GHz only after ~3us of uninterrupted work) and
    overlapping attention's scalar/vector load with projection's PE load.
  * Engine split: exp->scalar, masks+recip+qk-evict->vector, v-evict->
    scalar, normalize-evict->gpsimd, DMA issue split sync/gpsimd.
"""

import os
import sys

import numpy as np

if "/opt/trn_rl_repo" not in sys.path:
    sys.path.insert(0, "/opt/trn_rl_repo")

B, T, C = 2, 2048, 1024
HEADS = 16
D = C // HEADS  # 64
WINDOW = 128
N_CORES = 8
HPC = HEADS // 4  # heads per core (4)
OPC = 3 * HPC * D  # projection output rows per core (768)

PDT_NAME = os.environ.get("SA_PDT", "bf16")

_PROGRAM_CACHE = {}


def _build_program(pdt_name):
    import concourse.mybir as mybir
    from concourse import bacc
    import concourse.tile as tile
    from contextlib import ExitStack

    f32 = mybir.dt.float32
    f32r = mybir.dt.float32r
    bf16 = mybir.dt.bfloat16
    PDT = bf16 if pdt_name == "bf16" else f32r
    Exp = mybir.ActivationFunctionType.Exp

    nc = bacc.Bacc()
    xT_d = nc.declare_dram_parameter("xT", [C, T], bf16, isOutput=False)
    wT_d = nc.declare_dram_parameter("wT", [C, OPC], bf16, isOutput=False)
    msk_d = nc.declare_dram_parameter("msk", [128, 2, HPC, 128], bf16, isOutput=False)
    out_d = nc.declare_dram_parameter("out", [T, HPC * D], f32, isOutput=True)

    CC = C // 128  # 8 contraction chunks
    TS = 512  # projection t-slice
    NS = T // TS  # 4 slices
    NB = T // 128  # 16 key chunks / query blocks

    with ExitStack() as ctx:
        tc = ctx.enter_context(tile.TileContext(nc))
        const = ctx.enter_context(tc.tile_pool(name="const", bufs=1))
        xpool = ctx.enter_context(tc.tile_pool(name="xp", bufs=4))
        ppool = ctx.enter_context(tc.tile_pool(name="pp", bufs=5))
        rpool = ctx.enter_context(tc.tile_pool(name="rp", bufs=2))
        qk_ps = ctx.enter_context(tc.tile_pool(name="qkps", bufs=2, space="PSUM"))
        ov_ps = ctx.enter_context(tc.tile_pool(name="ovps", bufs=2, space="PSUM"))
        ps_const = ctx.enter_context(tc.tile_pool(name="psc", bufs=1, space="PSUM"))

        # ---- constants / persistent tiles ----
        w_sb = const.tile([128, CC, OPC], bf16)
        mskb = const.tile([128, 2, HPC, 128], PDT)
        q_sb = const.tile([128, 2, T], bf16)
        k_sb = const.tile([128, 2, T], bf16)
        v_sb = const.tile([128, NB, HPC, D + 1], PDT)
        o_sb = const.tile([128, NB, HPC * D], f32)
        wT_r = wT_d.rearrange("(cc p) o -> p cc o", p=128)

        # PSUM: qk 2 banks, sct4 4 banks, ov 2 banks = 8 total.
        # A matmul with start=True zeroes its whole 2KB PSUM bank ("zero
        # region"), so concurrent accumulators must each own a bank: sct4
        # pads each head's 384-col scores to a 512-col bank, and each ov
        # accumulation is a CONTIGUOUS 3-matmul chain into a bank-sized
        # rotating buffer (all three p^T chunk tiles are alive at once).
        sct4 = ps_const.tile([128, HPC, 512], f32)

        xT_r = xT_d.rearrange("(cc p) t -> p cc t", p=128)
        xs_tiles = {}

        def emit_dma_x(s):
            xs = xpool.tile([128, CC, TS], bf16, tag="xs", name=f"xs{s}")
            xs_tiles[s] = xs
            for c in range(CC):
                eng = nc.sync if c % 2 == 0 else nc.gpsimd
                eng.dma_start(out=xs[:, c, :], in_=xT_r[:, c, s * TS:(s + 1) * TS])

        # x slice 0 and weights fan out over FOUR issue queues (vector and
        # scalar are idle until attention starts), x0[c] and w[c] on
        # different queues so the first chain's pairs land in parallel.
        xs0 = xpool.tile([128, CC, TS], bf16, tag="xs", name="xs0")
        xs_tiles[0] = xs0
        for c in range(CC):
            eng_a = nc.sync if c % 2 == 0 else nc.gpsimd
            eng_b = nc.gpsimd if c % 2 == 0 else nc.sync
            eng_a.dma_start(out=xs0[:, c, :], in_=xT_r[:, c, 0:TS])
            eng_b.dma_start(out=w_sb[:, c, :], in_=wT_r[:, c, :])
        nc.sync.dma_start(out=mskb, in_=msk_d[:, :, :, :])
        nc.vector.memset(v_sb[:, :, :, D:D + 1], 1.0)

        def emit_proj_qk(s):
            xs = xs_tiles[s]
            for m in range(4):
                ps = qk_ps.tile([128, TS], f32, tag="ps", name=f"ps{s}_{m}")
                for c in range(CC):
                    nc.tensor.matmul(
                        ps,
                        lhsT=w_sb[:, c, m * 128:(m + 1) * 128],
                        rhs=xs[:, c, :],
                        start=(c == 0),
                        stop=(c == CC - 1),
                    )
                dst = (q_sb if m < 2 else k_sb)[:, m % 2, s * TS:(s + 1) * TS]
                nc.scalar.copy(dst, ps)

        def emit_proj_v(s, t4):
            xs = xs_tiles[s]
            pv = qk_ps.tile([128, TS], f32, tag="ps", name=f"pv{s}_{t4}")
            pv = pv[:, 0:HPC * D]
            for c in range(CC):
                nc.tensor.matmul(
                    pv,
                    lhsT=xs[:, c, t4 * 128:(t4 + 1) * 128],
                    rhs=w_sb[:, c, 2 * HPC * D:3 * HPC * D],
                    start=(c == 0),
                    stop=(c == CC - 1),
                )
            tb = s * 4 + t4
            nc.scalar.copy(
                v_sb[:, tb, :, 0:D], pv.rearrange("p (h d) -> p h d", h=HPC)
            )

        p_tiles = {}

        def emit_sc(jb):
            lo, hi = max(jb - 1, 0), min(jb + 1, NB - 1)
            width = (hi - lo + 1) * 128
            for h in range(HPC):
                mt, po = divmod(h, 2)
                po *= 64
                nc.tensor.matmul(
                    sct4[:, h, 0:width],
                    lhsT=k_sb[po:po + 64, mt, jb * 128:(jb + 1) * 128],
                    rhs=q_sb[po:po + 64, mt, lo * 128:(hi + 1) * 128],
                    start=True,
                    stop=True,
                )
            p_t = ppool.tile([128, HPC, 384], PDT, tag="p", name=f"p{jb}")
            p_tiles[jb] = p_t
            # pair-split: the next chunk's score matmuls WAR-wait on the exp
            # reading their sct bank, so finishing half the heads early
            # releases the PE half an exp sooner.
            for pr in range(2):
                hh = slice(2 * pr, 2 * pr + 2)
                nc.scalar.activation(
                    p_t[:, hh, 0:width], sct4[:, hh, 0:width], Exp
                )
            # edge thirds: seg of q=jb-1 needs (c >= r), seg of q=jb+1 (c <= r)
            off = (hi - lo) * 128
            if lo == jb - 1:
                nc.vector.tensor_mul(
                    p_t[:, :, 0:128], p_t[:, :, 0:128], mskb[:, 1, :, :]
                )
            if hi == jb + 1:
                nc.vector.tensor_mul(
                    p_t[:, :, off:off + 128], p_t[:, :, off:off + 128],
                    mskb[:, 0, :, :],
                )

        def emit_pv(q):
            # one contiguous accumulation chain per (q, h); the chain reads
            # p^T segments out of the (up to) three live p_t tiles.
            jbs = [jb for jb in (q - 1, q, q + 1) if 0 <= jb < NB]
            for h in range(HPC):
                ov = ov_ps.tile([128, 512], f32, tag="ov", name=f"ov{q}_{h}")
                for n, jb in enumerate(jbs):
                    seg = q - max(jb - 1, 0)
                    nc.tensor.matmul(
                        ov[:, 0:D + 1],
                        lhsT=p_tiles[jb][:, h, seg * 128:(seg + 1) * 128],
                        rhs=v_sb[:, jb, h, :],
                        start=(n == 0),
                        stop=(n == len(jbs) - 1),
                    )
                r_t = rpool.tile([128, 1], f32, tag=f"r{h}", name=f"r{q}_{h}")
                nc.vector.reciprocal(r_t, ov[:, D:D + 1])
                nc.vector.tensor_scalar_mul(
                    o_sb[:, q, h * D:(h + 1) * D], ov[:, 0:D], r_t
                )
            out_r = out_d.rearrange("(q p) o -> p q o", p=128)
            if q >= 12:
                # singles at the end so the final transfer is short
                nc.sync.dma_start(out=out_r[:, q:q + 1, :], in_=o_sb[:, q:q + 1, :])
            elif q % 4 == 3:
                nc.sync.dma_start(
                    out=out_r[:, q - 3:q + 1, :], in_=o_sb[:, q - 3:q + 1, :]
                )

        # ---- interleaved emission ----
        # chunk jb needs q/k of slice (jb+2)//4 and v block jb+1, so one new
        # chunk becomes runnable after each v t-block: jb = 4*s + t4 - 1.
        # PV chains lag score emission by 2 chunks so exp(jb) has a full
        # chunk of slack before the PE needs p_t(jb).
        emit_dma_x(1)
        emit_dma_x(2)
        for s in range(NS):
            emit_proj_qk(s)
            for t4 in range(4):
                emit_proj_v(s, t4)
                jb = 4 * s + t4 - 1
                if jb >= 0:
                    emit_sc(jb)
                    if jb >= 2:
                        emit_pv(jb - 2)
                        p_tiles.pop(jb - 3, None)
            if s == 0:
                emit_dma_x(3)
        emit_sc(NB - 1)
        for q in (NB - 3, NB - 2, NB - 1):
            emit_pv(q)

    nc.compile()
    return nc


def _host_inputs(x, Wqkv):
    """Per-core input maps: shard batch x head-group, pre-transpose, bf16."""
    import ml_dtypes

    bf16 = ml_dtypes.bfloat16
    scale = float(D) ** -0.5
    r = np.arange(128, dtype=np.float32)[:, None]
    ci = np.arange(128, dtype=np.float32)[None, :]
    # type 0: (c <= r)  applied to the seg of query block jb+1
    # type 1: (c >= r)  applied to the seg of query block jb-1
    msk = np.stack([(ci <= r), (ci >= r)], axis=1).astype(np.float32)  # [128,2,128]
    msk = np.repeat(msk[:, :, None, :], HPC, axis=2)  # [128, 2, HPC, 128]
    msk = np.ascontiguousarray(msk).astype(bf16)

    x = np.asarray(x, dtype=np.float32)
    Wqkv = np.asarray(Wqkv, dtype=np.float32)
    xT = [np.ascontiguousarray(x[b].T).astype(bf16) for b in range(B)]
    in_maps = []
    for core in range(N_CORES):
        b, hg = divmod(core, N_CORES // B)
        rows = slice(hg * HPC * D, (hg + 1) * HPC * D)
        wcat = np.concatenate(
            [
                Wqkv[0 * C:1 * C][rows] * scale,
                Wqkv[1 * C:2 * C][rows],
                Wqkv[2 * C:3 * C][rows],
            ],
            axis=0,
        )
        in_maps.append(
            {
                "xT": xT[b],
                "wT": np.ascontiguousarray(wcat.T).astype(bf16),
                "msk": msk,
            }
        )
    return in_maps


def _gather(results):
    out = np.empty((B, T, C), dtype=np.float32)
    for core in range(N_CORES):
        b, hg = divmod(core, N_CORES // B)
        out[b, :, hg * HPC * D:(hg + 1) * HPC * D] = results[core]["out"]
    return out


def kernel(x, Wqkv):
    from concourse.bass_utils import run_bass_kernel_spmd

    key = PDT_NAME
    if key not in _PROGRAM_CACHE:
        _PROGRAM_CACHE[key] = _build_program(key)
    nc = _PROGRAM_CACHE[key]
    in_maps = _host_inputs(x, Wqkv)
    res = run_bass_kernel_spmd(nc, in_maps, list(range(N_CORES)))
    return _gather(res.results)


# revision 54
# speedup vs baseline: 1.1517x; 1.0076x over previous
"""Trainium2 Bass kernel for nn_BaseAttention (sliding-window attention).

Full-input contract: kernel(x, Wqkv) -> [B, T, C] float32.

Sharding (8 cores): data-parallel over B (2) x tensor-parallel over head
groups (16 heads -> 4 groups of 4). Core c handles batch c//4, head group
c%4. Each core computes its QKV projection slice (768 of 3072 output rows)
and banded attention for its 4 heads; outputs are disjoint channel slices
of the final [B, T, C] tensor, so no collectives are needed.

Device-side design (per core), v2 -- engine-balanced and fully pipelined:

  * Projection (fp32r, all matmul free dims >= 256 so fp32r runs at full
    PE rate): q/k land transposed [d_part, T]; v lands [t_part, d] and is
    stored bf16 with a ones column appended ([v | 1]).
  * Attention runs KEY-chunk-major: for key chunk jb, ONE score matmul per
    head produces scores^T [128 keys, 384 queries] (query blocks jb-1..jb+1)
    -- free dim 384 >= 256 keeps fp32r at 1 cycle/row (the old per-block
    [128,128] scores ran at 1/4 rate).
  * exp on the scalar engine over two heads at once (strided AP) amortizes
    the ~220ns fixed access cost; sliding-window masking is two 0/1
    multiplies (all-4-heads wide, bf16/SBUF fast path) on the vector engine
    covering the two edge thirds only.
  * PV is FLIPPED: matmul(out[q,65], lhsT=p^T chunk, rhs=[v|1]) accumulates
    the output block in [query, d] orientation directly, so the old
    PSUM-copy + PE-transpose per (block, head) disappears entirely. Column
    64 is the softmax denominator l; normalize-and-evict is a per-partition
    tensor_scalar multiply with 1/l.
  * Emission interleaves projection slices with attention chunk groups
    (chunk jb only needs t < (jb+2)*128), keeping the PE continuously busy
    (it ramps 0.65->2.4 GHz only after ~3us of uninterrupted work) and
    overlapping attention's scalar/vector load with projection's PE load.
  * Engine split: exp->scalar, masks+recip+qk-evict->vector, v-evict->
    scalar, normalize-evict->gpsimd, DMA issue split sync/gpsimd.
"""

import os
import sys

import numpy as np

if "/opt/trn_rl_repo" not in sys.path:
    sys.path.insert(0, "/opt/trn_rl_repo")

B, T, C = 2, 2048, 1024
HEADS = 16
D = C // HEADS  # 64
WINDOW = 128
N_CORES = 8
HPC = HEADS // 4  # heads per core (4)
OPC = 3 * HPC * D  # projection output rows per core (768)

PDT_NAME = os.environ.get("SA_PDT", "bf16")

_PROGRAM_CACHE = {}


def _build_program(pdt_name):
    import concourse.mybir as mybir
    from concourse import bacc
    import concourse.tile as tile
    from contextlib import ExitStack

    f32 = mybir.dt.float32
    f32r = mybir.dt.float32r
    bf16 = mybir.dt.bfloat16
    PDT = bf16 if pdt_name == "bf16" else f32r
    Exp = mybir.ActivationFunctionType.Exp

    nc = bacc.Bacc()
    xT_d = nc.declare_dram_parameter("xT", [C, T], bf16, isOutput=False)
    wT_d = nc.declare_dram_parameter("wT", [C, OPC], bf16, isOutput=False)
    msk_d = nc.declare_dram_parameter("msk", [128, 2, HPC, 128], bf16, isOutput=False)
    out_d = nc.declare_dram_parameter("out", [T, HPC * D], f32, isOutput=True)

    CC = C // 128  # 8 contraction chunks
    TS = 512  # projection t-slice
    NS = T // TS  # 4 slices
    NB = T // 128  # 16 key chunks / query blocks

    with ExitStack() as ctx:
        tc = ctx.enter_context(tile.TileContext(nc))
        const = ctx.enter_context(tc.tile_pool(name="const", bufs=1))
        xpool = ctx.enter_context(tc.tile_pool(name="xp", bufs=4))
        ppool = ctx.enter_context(tc.tile_pool(name="pp", bufs=5))
        rpool = ctx.enter_context(tc.tile_pool(name="rp", bufs=2))
        qk_ps = ctx.enter_context(tc.tile_pool(name="qkps", bufs=2, space="PSUM"))
        ov_ps = ctx.enter_context(tc.tile_pool(name="ovps", bufs=2, space="PSUM"))
        ps_const = ctx.enter_context(tc.tile_pool(name="psc", bufs=1, space="PSUM"))

        # ---- constants / persistent tiles ----
        w_sb = const.tile([128, CC, OPC], bf16)
        mskb = const.tile([128, 2, HPC, 128], PDT)
        q_sb = const.tile([128, 2, T], bf16)
        k_sb = const.tile([128, 2, T], bf16)
        v_sb = const.tile([128, NB, HPC, D + 1], PDT)
        o_sb = const.tile([128, NB, HPC * D], f32)
        wT_r = wT_d.rearrange("(cc p) o -> p cc o", p=128)

        # PSUM: qk 2 banks, sct4 4 banks, ov 2 banks = 8 total.
        # A matmul with start=True zeroes its whole 2KB PSUM bank ("zero
        # region"), so concurrent accumulators must each own a bank: sct4
        # pads each head's 384-col scores to a 512-col bank, and each ov
        # accumulation is a CONTIGUOUS 3-matmul chain into a bank-sized
        # rotating buffer (all three p^T chunk tiles are alive at once).
        sct4 = ps_const.tile([128, HPC, 512], f32)

        xT_r = xT_d.rearrange("(cc p) t -> p cc t", p=128)
        xs_tiles = {}

        def emit_dma_x(s):
            xs = xpool.tile([128, CC, TS], bf16, tag="xs", name=f"xs{s}")
            xs_tiles[s] = xs
            for c in range(CC):
                eng = nc.sync if c % 2 == 0 else nc.gpsimd
                eng.dma_start(out=xs[:, c, :], in_=xT_r[:, c, s * TS:(s + 1) * TS])

        # x slice 0 and weights fan out over FOUR issue queues (vector and
        # scalar are idle until attention starts), x0[c] and w[c] on
        # different queues so the first chain's pairs land in parallel.
        xs0 = xpool.tile([128, CC, TS], bf16, tag="xs", name="xs0")
        xs_tiles[0] = xs0
        for c in range(CC):
            eng_a = nc.sync if c % 2 == 0 else nc.gpsimd
            eng_b = nc.gpsimd if c % 2 == 0 else nc.sync
            eng_a.dma_start(out=xs0[:, c, :], in_=xT_r[:, c, 0:TS])
            eng_b.dma_start(out=w_sb[:, c, :], in_=wT_r[:, c, :])
        nc.sync.dma_start(out=mskb, in_=msk_d[:, :, :, :])
        nc.vector.memset(v_sb[:, :, :, D:D + 1], 1.0)

        def emit_proj_qk(s):
            xs = xs_tiles[s]
            for m in range(4):
                ps = qk_ps.tile([128, TS], f32, tag="ps", name=f"ps{s}_{m}")
                for c in range(CC):
                    nc.tensor.matmul(
                        ps,
                        lhsT=w_sb[:, c, m * 128:(m + 1) * 128],
                        rhs=xs[:, c, :],
                        start=(c == 0),
                        stop=(c == CC - 1),
                    )
                dst = (q_sb if m < 2 else k_sb)[:, m % 2, s * TS:(s + 1) * TS]
                nc.scalar.copy(dst, ps)

        def emit_proj_v(s, t4):
            xs = xs_tiles[s]
            pv = qk_ps.tile([128, TS], f32, tag="ps", name=f"pv{s}_{t4}")
            pv = pv[:, 0:HPC * D]
            for c in range(CC):
                nc.tensor.matmul(
                    pv,
                    lhsT=xs[:, c, t4 * 128:(t4 + 1) * 128],
                    rhs=w_sb[:, c, 2 * HPC * D:3 * HPC * D],
                    start=(c == 0),
                    stop=(c == CC - 1),
                )
            tb = s * 4 + t4
            nc.scalar.copy(
                v_sb[:, tb, :, 0:D], pv.rearrange("p (h d) -> p h d", h=HPC)
            )

        p_tiles = {}

        def emit_sc(jb):
            lo, hi = max(jb - 1, 0), min(jb + 1, NB - 1)
            width = (hi - lo + 1) * 128
            for h in range(HPC):
                mt, po = divmod(h, 2)
                po *= 64
                nc.tensor.matmul(
                    sct4[:, h, 0:width],
                    lhsT=k_sb[po:po + 64, mt, jb * 128:(jb + 1) * 128],
                    rhs=q_sb[po:po + 64, mt, lo * 128:(hi + 1) * 128],
                    start=True,
                    stop=True,
                )
            p_t = ppool.tile([128, HPC, 384], PDT, tag="p", name=f"p{jb}")
            p_tiles[jb] = p_t
            if jb >= 12:
                # post-projection chunks: the scalar queue is exp-only here,
                # so pair-split releases the sct-bank WAR half a chunk sooner.
                for pr in range(2):
                    hh = slice(2 * pr, 2 * pr + 2)
                    nc.scalar.activation(
                        p_t[:, hh, 0:width], sct4[:, hh, 0:width], Exp
                    )
            else:
                nc.scalar.activation(p_t[:, :, 0:width], sct4[:, :, 0:width], Exp)
            # edge thirds: seg of q=jb-1 needs (c >= r), seg of q=jb+1 (c <= r)
            off = (hi - lo) * 128
            if lo == jb - 1:
                nc.vector.tensor_mul(
                    p_t[:, :, 0:128], p_t[:, :, 0:128], mskb[:, 1, :, :]
                )
            if hi == jb + 1:
                nc.vector.tensor_mul(
                    p_t[:, :, off:off + 128], p_t[:, :, off:off + 128],
                    mskb[:, 0, :, :],
                )

        def emit_pv(q):
            # one contiguous accumulation chain per (q, h); the chain reads
            # p^T segments out of the (up to) three live p_t tiles.
            jbs = [jb for jb in (q - 1, q, q + 1) if 0 <= jb < NB]
            for h in range(HPC):
                ov = ov_ps.tile([128, 512], f32, tag="ov", name=f"ov{q}_{h}")
                for n, jb in enumerate(jbs):
                    seg = q - max(jb - 1, 0)
                    nc.tensor.matmul(
                        ov[:, 0:D + 1],
                        lhsT=p_tiles[jb][:, h, seg * 128:(seg + 1) * 128],
                        rhs=v_sb[:, jb, h, :],
                        start=(n == 0),
                        stop=(n == len(jbs) - 1),
                    )
                r_t = rpool.tile([128, 1], f32, tag=f"r{h}", name=f"r{q}_{h}")
                nc.vector.reciprocal(r_t, ov[:, D:D + 1])
                nc.vector.tensor_scalar_mul(
                    o_sb[:, q, h * D:(h + 1) * D], ov[:, 0:D], r_t
                )
            out_r = out_d.rearrange("(q p) o -> p q o", p=128)
            if q >= 12:
                # singles at the end so the final transfer is short
                nc.sync.dma_start(out=out_r[:, q:q + 1, :], in_=o_sb[:, q:q + 1, :])
            elif q % 4 == 3:
                nc.sync.dma_start(
                    out=out_r[:, q - 3:q + 1, :], in_=o_sb[:, q - 3:q + 1, :]
                )

        # ---- interleaved emission ----
        # chunk jb needs q/k of slice (jb+2)//4 and v block jb+1, so one new
        # chunk becomes runnable after each v t-block: jb = 4*s + t4 - 1.
        # PV chains lag score emission by 2 chunks so exp(jb) has a full
        # chunk of slack before the PE needs p_t(jb).
        emit_dma_x(1)
        emit_dma_x(2)
        for s in range(NS):
            emit_proj_qk(s)
            for t4 in range(4):
                emit_proj_v(s, t4)
                jb = 4 * s + t4 - 1
                if jb >= 0:
                    emit_sc(jb)
                    if jb >= 2:
                        emit_pv(jb - 2)
                        p_tiles.pop(jb - 3, None)
            if s == 0:
                emit_dma_x(3)
        emit_sc(NB - 1)
        for q in (NB - 3, NB - 2, NB - 1):
            emit_pv(q)

    nc.compile()
    return nc


def _host_inputs(x, Wqkv):
    """Per-core input maps: shard batch x head-group, pre-transpose, bf16."""
    import ml_dtypes

    bf16 = ml_dtypes.bfloat16
    scale = float(D) ** -0.5
    r = np.arange(128, dtype=np.float32)[:, None]
    ci = np.arange(128, dtype=np.float32)[None, :]
    # type 0: (c <= r)  applied to the seg of query block jb+1
    # type 1: (c >= r)  applied to the seg of query block jb-1
    msk = np.stack([(ci <= r), (ci >= r)], axis=1).astype(np.float32)  # [128,2,128]
    msk = np.repeat(msk[:, :, None, :], HPC, axis=2)  # [128, 2, HPC, 128]
    msk = np.ascontiguousarray(msk).astype(bf16)

    x = np.asarray(x, dtype=np.float32)
    Wqkv = np.asarray(Wqkv, dtype=np.float32)
    xT = [np.ascontiguousarray(x[b].T).astype(bf16) for b in range(B)]
    in_maps = []
    for core in range(N_CORES):
        b, hg = divmod(core, N_CORES // B)
        rows = slice(hg * HPC * D, (hg + 1) * HPC * D)
        wcat = np.concatenate(
            [
                Wqkv[0 * C:1 * C][rows] * scale,
                Wqkv[1 * C:2 * C][rows],
                Wqkv[2 * C:3 * C][rows],
            ],
            axis=0,
        )
        in_maps.append(
            {
                "xT": xT[b],
                "wT": np.ascontiguousarray(wcat.T).astype(bf16),
                "msk": msk,
            }
        )
    return in_maps


def _gather(results):
    out = np.empty((B, T, C), dtype=np.float32)
    for core in range(N_CORES):
        b, hg = divmod(core, N_CORES // B)
        out[b, :, hg * HPC * D:(hg + 1) * HPC * D] = results[core]["out"]
    return out


def kernel(x, Wqkv):
    from concourse.bass_utils import run_bass_kernel_spmd

    key = PDT_NAME
    if key not in _PROGRAM_CACHE:
        _PROGRAM_CACHE[key] = _build_program(key)
    nc = _PROGRAM_CACHE[key]
    in_maps = _host_inputs(x, Wqkv)
    res = run_bass_kernel_spmd(nc, in_maps, list(range(N_CORES)))
    return _gather(res.results)


# revision 56
# speedup vs baseline: 1.1642x; 1.0109x over previous
"""Trainium2 Bass kernel for nn_BaseAttention (sliding-window attention).

Full-input contract: kernel(x, Wqkv) -> [B, T, C] float32.

Sharding (8 cores): data-parallel over B (2) x tensor-parallel over head
groups (16 heads -> 4 groups of 4). Core c handles batch c//4, head group
c%4. Each core computes its QKV projection slice (768 of 3072 output rows)
and banded attention for its 4 heads; outputs are disjoint channel slices
of the final [B, T, C] tensor, so no collectives are needed.

Device-side design (per core) -- engine-balanced and fully pipelined:

  * Everything bf16 except PSUM accumulation and the final output (f32):
    halves HBM traffic and PE weight-load time; rel err ~8e-3 vs the 2e-2
    gate.
  * Projection: q/k land transposed [d_part, T] via 512-col m-tile chains;
    v lands [t_part, d] and is stored with a ones column appended ([v | 1]).
  * Attention runs KEY-chunk-major: for key chunk jb, ONE score matmul per
    head produces scores^T [128 keys, 384 queries] (query blocks jb-1..jb+1)
    instead of three 128-col matmuls.
  * exp on the scalar engine over all 4 heads at once (strided AP over the
    four score banks) amortizes the ~220ns fixed access cost; sliding-window
    masking is two 0/1 multiplies (all-4-heads wide, bf16/SBUF) on the
    vector engine covering the two edge thirds only.
  * PV is FLIPPED: matmul(out[q,65], lhsT=p^T chunk, rhs=[v|1]) accumulates
    the output block in [query, d] orientation directly, so no PSUM-copy +
    PE-transpose per (block, head) is needed. Column 64 is the softmax
    denominator l; normalize-and-evict is a reciprocal + per-partition
    tensor_scalar multiply on the vector engine.
  * PSUM discipline: a start=True matmul zeroes its whole 2KB bank ("zero
    region"), so concurrent accumulators each own a bank: 2 banks for
    projection psum (double-buffered), 4 for the per-head score banks
    (padded 384->512), 2 rotating bank-sized PV accumulators written by
    CONTIGUOUS 3-matmul chains (all three p^T tiles are alive at once).
  * Emission interleaves projection with attention at v-t-block granularity
    (chunk jb needs q/k of slice (jb+2)//4 and v block jb+1), keeping the
    PE continuously busy (it ramps 0.65->2.4 GHz only after ~3us of
    uninterrupted work) and overlapping attention's scalar/vector load with
    projection's PE load; only chunk 15 + three PV flushes trail the last
    projection matmul.
  * DMA: per-chunk descriptors, x-slice-0 and W interleaved on opposite
    sync/gpsimd issue queues (each engine owns ONE serial DMA queue);
    output DMA'd in 4-block batches, singles at the end.
"""

import os
import sys

import numpy as np

if "/opt/trn_rl_repo" not in sys.path:
    sys.path.insert(0, "/opt/trn_rl_repo")

B, T, C = 2, 2048, 1024
HEADS = 16
D = C // HEADS  # 64
WINDOW = 128
N_CORES = 8
HPC = HEADS // 4  # heads per core (4)
OPC = 3 * HPC * D  # projection output rows per core (768)

PDT_NAME = os.environ.get("SA_PDT", "bf16")

_PROGRAM_CACHE = {}


def _build_program(pdt_name):
    import concourse.mybir as mybir
    from concourse import bacc
    import concourse.tile as tile
    from contextlib import ExitStack

    f32 = mybir.dt.float32
    f32r = mybir.dt.float32r
    bf16 = mybir.dt.bfloat16
    PDT = bf16 if pdt_name == "bf16" else f32r
    Exp = mybir.ActivationFunctionType.Exp

    nc = bacc.Bacc()
    xT_d = nc.declare_dram_parameter("xT", [C, T], bf16, isOutput=False)
    wT_d = nc.declare_dram_parameter("wT", [C, OPC], bf16, isOutput=False)
    msk_d = nc.declare_dram_parameter("msk", [128, 2, HPC, 128], bf16, isOutput=False)
    out_d = nc.declare_dram_parameter("out", [T, HPC * D], f32, isOutput=True)

    CC = C // 128  # 8 contraction chunks
    TS = 512  # projection t-slice
    NS = T // TS  # 4 slices
    NB = T // 128  # 16 key chunks / query blocks

    with ExitStack() as ctx:
        tc = ctx.enter_context(tile.TileContext(nc))
        const = ctx.enter_context(tc.tile_pool(name="const", bufs=1))
        xpool = ctx.enter_context(tc.tile_pool(name="xp", bufs=4))
        ppool = ctx.enter_context(tc.tile_pool(name="pp", bufs=5))
        rpool = ctx.enter_context(tc.tile_pool(name="rp", bufs=2))
        qk_ps = ctx.enter_context(tc.tile_pool(name="qkps", bufs=2, space="PSUM"))
        ov_ps = ctx.enter_context(tc.tile_pool(name="ovps", bufs=2, space="PSUM"))
        ps_const = ctx.enter_context(tc.tile_pool(name="psc", bufs=1, space="PSUM"))

        # ---- constants / persistent tiles ----
        w_sb = const.tile([128, CC, OPC], bf16)
        mskb = const.tile([128, 2, HPC, 128], PDT)
        q_sb = const.tile([128, 2, T], bf16)
        k_sb = const.tile([128, 2, T], bf16)
        v_sb = const.tile([128, NB, HPC, D + 1], PDT)
        o_sb = const.tile([128, NB, HPC * D], f32)
        wT_r = wT_d.rearrange("(cc p) o -> p cc o", p=128)

        # PSUM: qk 2 banks, sct4 4 banks, ov 2 banks = 8 total.
        # A matmul with start=True zeroes its whole 2KB PSUM bank ("zero
        # region"), so concurrent accumulators must each own a bank: sct4
        # pads each head's 384-col scores to a 512-col bank, and each ov
        # accumulation is a CONTIGUOUS 3-matmul chain into a bank-sized
        # rotating buffer (all three p^T chunk tiles are alive at once).
        sct4 = ps_const.tile([128, HPC, 512], f32)

        xT_r = xT_d.rearrange("(cc p) t -> p cc t", p=128)
        xs_tiles = {}

        def emit_dma_x(s):
            xs = xpool.tile([128, CC, TS], bf16, tag="xs", name=f"xs{s}")
            xs_tiles[s] = xs
            for c in range(CC):
                eng = nc.sync if c % 2 == 0 else nc.gpsimd
                eng.dma_start(out=xs[:, c, :], in_=xT_r[:, c, s * TS:(s + 1) * TS])

        # x slice 0 and weights interleaved chunk-by-chunk, x0[c] and w[c]
        # on OPPOSITE queues so the first chain's pairs land in parallel.
        xs0 = xpool.tile([128, CC, TS], bf16, tag="xs", name="xs0")
        xs_tiles[0] = xs0
        for c in range(CC):
            eng_a = nc.sync if c % 2 == 0 else nc.gpsimd
            eng_b = nc.gpsimd if c % 2 == 0 else nc.sync
            eng_a.dma_start(out=xs0[:, c, :], in_=xT_r[:, c, 0:TS])
            eng_b.dma_start(out=w_sb[:, c, :], in_=wT_r[:, c, :])
        nc.sync.dma_start(out=mskb, in_=msk_d[:, :, :, :])
        nc.vector.memset(v_sb[:, :, :, D:D + 1], 1.0)

        def emit_proj_qk(s):
            xs = xs_tiles[s]
            for m in range(4):
                ps = qk_ps.tile([128, TS], f32, tag="ps", name=f"ps{s}_{m}")
                for c in range(CC):
                    nc.tensor.matmul(
                        ps,
                        lhsT=w_sb[:, c, m * 128:(m + 1) * 128],
                        rhs=xs[:, c, :],
                        start=(c == 0),
                        stop=(c == CC - 1),
                    )
                dst = (q_sb if m < 2 else k_sb)[:, m % 2, s * TS:(s + 1) * TS]
                nc.scalar.copy(dst, ps)

        def emit_proj_v(s, t4):
            xs = xs_tiles[s]
            pv = qk_ps.tile([128, TS], f32, tag="ps", name=f"pv{s}_{t4}")
            pv = pv[:, 0:HPC * D]
            for c in range(CC):
                nc.tensor.matmul(
                    pv,
                    lhsT=xs[:, c, t4 * 128:(t4 + 1) * 128],
                    rhs=w_sb[:, c, 2 * HPC * D:3 * HPC * D],
                    start=(c == 0),
                    stop=(c == CC - 1),
                )
            tb = s * 4 + t4
            nc.scalar.copy(
                v_sb[:, tb, :, 0:D], pv.rearrange("p (h d) -> p h d", h=HPC)
            )

        p_tiles = {}

        def emit_sc(jb):
            lo, hi = max(jb - 1, 0), min(jb + 1, NB - 1)
            width = (hi - lo + 1) * 128
            for h in range(HPC):
                mt, po = divmod(h, 2)
                po *= 64
                nc.tensor.matmul(
                    sct4[:, h, 0:width],
                    lhsT=k_sb[po:po + 64, mt, jb * 128:(jb + 1) * 128],
                    rhs=q_sb[po:po + 64, mt, lo * 128:(hi + 1) * 128],
                    start=True,
                    stop=True,
                )
            p_t = ppool.tile([128, HPC, 384], PDT, tag="p", name=f"p{jb}")
            p_tiles[jb] = p_t
            if jb >= 12:
                # post-projection chunks: the scalar queue is exp-only here,
                # so pair-split releases the sct-bank WAR half a chunk sooner.
                for pr in range(2):
                    hh = slice(2 * pr, 2 * pr + 2)
                    nc.scalar.activation(
                        p_t[:, hh, 0:width], sct4[:, hh, 0:width], Exp
                    )
            else:
                nc.scalar.activation(p_t[:, :, 0:width], sct4[:, :, 0:width], Exp)
            # edge thirds: seg of q=jb-1 needs (c >= r), seg of q=jb+1 (c <= r)
            off = (hi - lo) * 128
            if lo == jb - 1:
                nc.vector.tensor_mul(
                    p_t[:, :, 0:128], p_t[:, :, 0:128], mskb[:, 1, :, :]
                )
            if hi == jb + 1:
                nc.vector.tensor_mul(
                    p_t[:, :, off:off + 128], p_t[:, :, off:off + 128],
                    mskb[:, 0, :, :],
                )

        def emit_pv(q):
            # one contiguous accumulation chain per (q, h); the chain reads
            # p^T segments out of the (up to) three live p_t tiles.
            jbs = [jb for jb in (q - 1, q, q + 1) if 0 <= jb < NB]
            for h in range(HPC):
                ov = ov_ps.tile([128, 512], f32, tag="ov", name=f"ov{q}_{h}")
                for n, jb in enumerate(jbs):
                    seg = q - max(jb - 1, 0)
                    nc.tensor.matmul(
                        ov[:, 0:D + 1],
                        lhsT=p_tiles[jb][:, h, seg * 128:(seg + 1) * 128],
                        rhs=v_sb[:, jb, h, :],
                        start=(n == 0),
                        stop=(n == len(jbs) - 1),
                    )
                r_t = rpool.tile([128, 1], f32, tag=f"r{h}", name=f"r{q}_{h}")
                nc.vector.reciprocal(r_t, ov[:, D:D + 1])
                nc.vector.tensor_scalar_mul(
                    o_sb[:, q, h * D:(h + 1) * D], ov[:, 0:D], r_t
                )
            out_r = out_d.rearrange("(q p) o -> p q o", p=128)
            if q >= 12:
                # singles at the end so the final transfer is short
                nc.sync.dma_start(out=out_r[:, q:q + 1, :], in_=o_sb[:, q:q + 1, :])
            elif q % 4 == 3:
                nc.sync.dma_start(
                    out=out_r[:, q - 3:q + 1, :], in_=o_sb[:, q - 3:q + 1, :]
                )

        # ---- interleaved emission ----
        # chunk jb needs q/k of slice (jb+2)//4 and v block jb+1, so one new
        # chunk becomes runnable after each v t-block: jb = 4*s + t4 - 1.
        # PV chains lag score emission by 2 chunks so exp(jb) has a full
        # chunk of slack before the PE needs p_t(jb).
        emit_dma_x(1)
        emit_dma_x(2)
        for s in range(NS):
            emit_proj_qk(s)
            for t4 in range(4):
                emit_proj_v(s, t4)
                jb = 4 * s + t4 - 1
                if jb >= 0:
                    emit_sc(jb)
                    if jb >= 2:
                        emit_pv(jb - 2)
                        p_tiles.pop(jb - 3, None)
            if s == 0:
                emit_dma_x(3)
        emit_sc(NB - 1)
        for q in (NB - 3, NB - 2, NB - 1):
            emit_pv(q)

    nc.compile()
    return nc


def _host_inputs(x, Wqkv):
    """Per-core input maps: shard batch x head-group, pre-transpose, bf16."""
    import ml_dtypes

    bf16 = ml_dtypes.bfloat16
    scale = float(D) ** -0.5
    r = np.arange(128, dtype=np.float32)[:, None]
    ci = np.arange(128, dtype=np.float32)[None, :]
    # type 0: (c <= r)  applied to the seg of query block jb+1
    # type 1: (c >= r)  applied to the seg of query block jb-1
    msk = np.stack([(ci <= r), (ci >= r)], axis=1).astype(np.float32)  # [128,2,128]
    msk = np.repeat(msk[:, :, None, :], HPC, axis=2)  # [128, 2, HPC, 128]
    msk = np.ascontiguousarray(msk).astype(bf16)

    x = np.asarray(x, dtype=np.float32)
    Wqkv = np.asarray(Wqkv, dtype=np.float32)
    xT = [np.ascontiguousarray(x[b].T).astype(bf16) for b in range(B)]
    in_maps = []
    for core in range(N_CORES):
        b, hg = divmod(core, N_CORES // B)
        rows = slice(hg * HPC * D, (hg + 1) * HPC * D)
        wcat = np.concatenate(
            [
                Wqkv[0 * C:1 * C][rows] * scale,
                Wqkv[1 * C:2 * C][rows],
                Wqkv[2 * C:3 * C][rows],
            ],
            axis=0,
        )
        in_maps.append(
            {
                "xT": xT[b],
                "wT": np.ascontiguousarray(wcat.T).astype(bf16),
                "msk": msk,
            }
        )
    return in_maps


def _gather(results):
    out = np.empty((B, T, C), dtype=np.float32)
    for core in range(N_CORES):
        b, hg = divmod(core, N_CORES // B)
        out[b, :, hg * HPC * D:(hg + 1) * HPC * D] = results[core]["out"]
    return out


def kernel(x, Wqkv):
    from concourse.bass_utils import run_bass_kernel_spmd

    key = PDT_NAME
    if key not in _PROGRAM_CACHE:
        _PROGRAM_CACHE[key] = _build_program(key)
    nc = _PROGRAM_CACHE[key]
    in_maps = _host_inputs(x, Wqkv)
    res = run_bass_kernel_spmd(nc, in_maps, list(range(N_CORES)))
    return _gather(res.results)


# revision 57
# speedup vs baseline: 1.1884x; 1.0207x over previous
"""Trainium2 Bass kernel for nn_BaseAttention (sliding-window attention).

Full-input contract: kernel(x, Wqkv) -> [B, T, C] float32.

Sharding (8 cores): data-parallel over B (2) x tensor-parallel over head
groups (16 heads -> 4 groups of 4). Core c handles batch c//4, head group
c%4. Each core computes its QKV projection slice (768 of 3072 output rows)
and banded attention for its 4 heads; outputs are disjoint channel slices
of the final [B, T, C] tensor, so no collectives are needed.

Device-side design (per core) -- engine-balanced and fully pipelined:

  * Everything bf16 except PSUM accumulation and the final output (f32):
    halves HBM traffic and PE weight-load time; rel err ~8e-3 vs the 2e-2
    gate.
  * Projection: q/k land transposed [d_part, T] via 512-col m-tile chains;
    v lands [t_part, d] and is stored with a ones column appended ([v | 1]).
  * Attention runs KEY-chunk-major: for key chunk jb, ONE score matmul per
    head produces scores^T [128 keys, 384 queries] (query blocks jb-1..jb+1)
    instead of three 128-col matmuls.
  * exp on the scalar engine over all 4 heads at once (strided AP over the
    four score banks) amortizes the ~220ns fixed access cost; sliding-window
    masking is two 0/1 multiplies (all-4-heads wide, bf16/SBUF) on the
    vector engine covering the two edge thirds only.
  * PV is FLIPPED: matmul(out[q,65], lhsT=p^T chunk, rhs=[v|1]) accumulates
    the output block in [query, d] orientation directly, so no PSUM-copy +
    PE-transpose per (block, head) is needed. Column 64 is the softmax
    denominator l; normalize-and-evict is a reciprocal + per-partition
    tensor_scalar multiply on the vector engine.
  * PSUM discipline: a start=True matmul zeroes its whole 2KB bank ("zero
    region"), so concurrent accumulators each own a bank: 2 banks for
    projection psum (double-buffered), 4 for the per-head score banks
    (padded 384->512), 2 rotating bank-sized PV accumulators written by
    CONTIGUOUS 3-matmul chains (all three p^T tiles are alive at once).
  * Emission interleaves projection with attention at v-t-block granularity
    (chunk jb needs q/k of slice (jb+2)//4 and v block jb+1), keeping the
    PE continuously busy (it ramps 0.65->2.4 GHz only after ~3us of
    uninterrupted work) and overlapping attention's scalar/vector load with
    projection's PE load; only chunk 15 + three PV flushes trail the last
    projection matmul.
  * DMA: per-chunk descriptors, x-slice-0 and W interleaved on opposite
    sync/gpsimd issue queues (each engine owns ONE serial DMA queue);
    output DMA'd in 4-block batches, singles at the end.
"""

import os
import sys

import numpy as np

if "/opt/trn_rl_repo" not in sys.path:
    sys.path.insert(0, "/opt/trn_rl_repo")

B, T, C = 2, 2048, 1024
HEADS = 16
D = C // HEADS  # 64
WINDOW = 128
N_CORES = 8
HPC = HEADS // 4  # heads per core (4)
OPC = 3 * HPC * D  # projection output rows per core (768)

PDT_NAME = os.environ.get("SA_PDT", "bf16")

_PROGRAM_CACHE = {}


def _build_program(pdt_name):
    import concourse.mybir as mybir
    from concourse import bacc
    import concourse.tile as tile
    from contextlib import ExitStack

    f32 = mybir.dt.float32
    f32r = mybir.dt.float32r
    bf16 = mybir.dt.bfloat16
    PDT = bf16 if pdt_name == "bf16" else f32r
    Exp = mybir.ActivationFunctionType.Exp

    nc = bacc.Bacc()
    xT_d = nc.declare_dram_parameter("xT", [C, T], bf16, isOutput=False)
    wT_d = nc.declare_dram_parameter("wT", [C, OPC], bf16, isOutput=False)
    msk_d = nc.declare_dram_parameter("msk", [128, 2, HPC, 128], bf16, isOutput=False)
    out_d = nc.declare_dram_parameter("out", [T, HPC * D], f32, isOutput=True)

    CC = C // 128  # 8 contraction chunks
    TS = 512  # projection t-slice
    NS = T // TS  # 4 slices
    NB = T // 128  # 16 key chunks / query blocks

    with ExitStack() as ctx:
        tc = ctx.enter_context(tile.TileContext(nc))
        const = ctx.enter_context(tc.tile_pool(name="const", bufs=1))
        xpool = ctx.enter_context(tc.tile_pool(name="xp", bufs=4))
        ppool = ctx.enter_context(tc.tile_pool(name="pp", bufs=5))
        rpool = ctx.enter_context(tc.tile_pool(name="rp", bufs=2))
        qk_ps = ctx.enter_context(tc.tile_pool(name="qkps", bufs=2, space="PSUM"))
        ov_ps = ctx.enter_context(tc.tile_pool(name="ovps", bufs=2, space="PSUM"))
        ps_const = ctx.enter_context(tc.tile_pool(name="psc", bufs=1, space="PSUM"))

        # ---- constants / persistent tiles ----
        w_sb = const.tile([128, CC, OPC], bf16)
        mskb = const.tile([128, 2, HPC, 128], PDT)
        q_sb = const.tile([128, 2, T], bf16)
        k_sb = const.tile([128, 2, T], bf16)
        v_sb = const.tile([128, NB, HPC, D + 1], PDT)
        o_sb = const.tile([128, NB, HPC * D], f32)
        wT_r = wT_d.rearrange("(cc p) o -> p cc o", p=128)

        # PSUM: qk 2 banks, sct4 4 banks, ov 2 banks = 8 total.
        # A matmul with start=True zeroes its whole 2KB PSUM bank ("zero
        # region"), so concurrent accumulators must each own a bank: sct4
        # pads each head's 384-col scores to a 512-col bank, and each ov
        # accumulation is a CONTIGUOUS 3-matmul chain into a bank-sized
        # rotating buffer (all three p^T chunk tiles are alive at once).
        sct4 = ps_const.tile([128, HPC, 512], f32)

        xT_r = xT_d.rearrange("(cc p) t -> p cc t", p=128)
        xs_tiles = {}

        def emit_dma_x(s):
            xs = xpool.tile([128, CC, TS], bf16, tag="xs", name=f"xs{s}")
            xs_tiles[s] = xs
            for c in range(CC):
                eng = nc.sync if c % 2 == 0 else nc.gpsimd
                eng.dma_start(out=xs[:, c, :], in_=xT_r[:, c, s * TS:(s + 1) * TS])

        # x slice 0 and weights interleaved chunk-by-chunk, x0[c] and w[c]
        # on OPPOSITE queues so the first chain's pairs land in parallel.
        xs0 = xpool.tile([128, CC, TS], bf16, tag="xs", name="xs0")
        xs_tiles[0] = xs0
        for c in range(CC):
            eng_a = nc.sync if c % 2 == 0 else nc.gpsimd
            eng_b = nc.gpsimd if c % 2 == 0 else nc.sync
            eng_a.dma_start(out=xs0[:, c, :], in_=xT_r[:, c, 0:TS])
            eng_b.dma_start(out=w_sb[:, c, :], in_=wT_r[:, c, :])
        nc.sync.dma_start(out=mskb, in_=msk_d[:, :, :, :])
        nc.vector.memset(v_sb[:, :, :, D:D + 1], 1.0)

        # warm the PE while the first DMAs land: the clock ramps
        # 0.65->1.2->2.4 GHz only under continuous load, so burn the DMA
        # wait on dummy matmuls over memset scratch (no data dependency).
        warm = const.tile([128, 640], bf16)
        nc.vector.memset(warm, 0.0)
        wps = qk_ps.tile([128, TS], f32, tag="ps", name="wps")
        for n in range(8):
            nc.tensor.matmul(
                wps,
                lhsT=warm[:, 0:128],
                rhs=warm[:, 128:640],
                start=(n == 0),
                stop=(n == 7),
            )

        def emit_proj_qk(s):
            xs = xs_tiles[s]
            for m in range(4):
                ps = qk_ps.tile([128, TS], f32, tag="ps", name=f"ps{s}_{m}")
                for c in range(CC):
                    nc.tensor.matmul(
                        ps,
                        lhsT=w_sb[:, c, m * 128:(m + 1) * 128],
                        rhs=xs[:, c, :],
                        start=(c == 0),
                        stop=(c == CC - 1),
                    )
                dst = (q_sb if m < 2 else k_sb)[:, m % 2, s * TS:(s + 1) * TS]
                nc.scalar.copy(dst, ps)

        def emit_proj_v(s, t4):
            xs = xs_tiles[s]
            pv = qk_ps.tile([128, TS], f32, tag="ps", name=f"pv{s}_{t4}")
            pv = pv[:, 0:HPC * D]
            for c in range(CC):
                nc.tensor.matmul(
                    pv,
                    lhsT=xs[:, c, t4 * 128:(t4 + 1) * 128],
                    rhs=w_sb[:, c, 2 * HPC * D:3 * HPC * D],
                    start=(c == 0),
                    stop=(c == CC - 1),
                )
            tb = s * 4 + t4
            nc.scalar.copy(
                v_sb[:, tb, :, 0:D], pv.rearrange("p (h d) -> p h d", h=HPC)
            )

        p_tiles = {}

        def emit_sc(jb):
            lo, hi = max(jb - 1, 0), min(jb + 1, NB - 1)
            width = (hi - lo + 1) * 128
            for h in range(HPC):
                mt, po = divmod(h, 2)
                po *= 64
                nc.tensor.matmul(
                    sct4[:, h, 0:width],
                    lhsT=k_sb[po:po + 64, mt, jb * 128:(jb + 1) * 128],
                    rhs=q_sb[po:po + 64, mt, lo * 128:(hi + 1) * 128],
                    start=True,
                    stop=True,
                )
            p_t = ppool.tile([128, HPC, 384], PDT, tag="p", name=f"p{jb}")
            p_tiles[jb] = p_t
            if jb >= 12:
                # post-projection chunks: the scalar queue is exp-only here,
                # so pair-split releases the sct-bank WAR half a chunk sooner.
                for pr in range(2):
                    hh = slice(2 * pr, 2 * pr + 2)
                    nc.scalar.activation(
                        p_t[:, hh, 0:width], sct4[:, hh, 0:width], Exp
                    )
            else:
                nc.scalar.activation(p_t[:, :, 0:width], sct4[:, :, 0:width], Exp)
            # edge thirds: seg of q=jb-1 needs (c >= r), seg of q=jb+1 (c <= r)
            off = (hi - lo) * 128
            if lo == jb - 1:
                nc.vector.tensor_mul(
                    p_t[:, :, 0:128], p_t[:, :, 0:128], mskb[:, 1, :, :]
                )
            if hi == jb + 1:
                nc.vector.tensor_mul(
                    p_t[:, :, off:off + 128], p_t[:, :, off:off + 128],
                    mskb[:, 0, :, :],
                )

        def emit_pv(q):
            # one contiguous accumulation chain per (q, h); the chain reads
            # p^T segments out of the (up to) three live p_t tiles.
            jbs = [jb for jb in (q - 1, q, q + 1) if 0 <= jb < NB]
            for h in range(HPC):
                ov = ov_ps.tile([128, 512], f32, tag="ov", name=f"ov{q}_{h}")
                for n, jb in enumerate(jbs):
                    seg = q - max(jb - 1, 0)
                    nc.tensor.matmul(
                        ov[:, 0:D + 1],
                        lhsT=p_tiles[jb][:, h, seg * 128:(seg + 1) * 128],
                        rhs=v_sb[:, jb, h, :],
                        start=(n == 0),
                        stop=(n == len(jbs) - 1),
                    )
                r_t = rpool.tile([128, 1], f32, tag=f"r{h}", name=f"r{q}_{h}")
                nc.vector.reciprocal(r_t, ov[:, D:D + 1])
                nc.vector.tensor_scalar_mul(
                    o_sb[:, q, h * D:(h + 1) * D], ov[:, 0:D], r_t
                )
            out_r = out_d.rearrange("(q p) o -> p q o", p=128)
            if q >= 12:
                # singles at the end so the final transfer is short
                nc.sync.dma_start(out=out_r[:, q:q + 1, :], in_=o_sb[:, q:q + 1, :])
            elif q % 4 == 3:
                nc.sync.dma_start(
                    out=out_r[:, q - 3:q + 1, :], in_=o_sb[:, q - 3:q + 1, :]
                )

        # ---- interleaved emission ----
        # chunk jb needs q/k of slice (jb+2)//4 and v block jb+1, so one new
        # chunk becomes runnable after each v t-block: jb = 4*s + t4 - 1.
        # PV chains lag score emission by 2 chunks so exp(jb) has a full
        # chunk of slack before the PE needs p_t(jb).
        emit_dma_x(1)
        emit_dma_x(2)
        for s in range(NS):
            emit_proj_qk(s)
            for t4 in range(4):
                emit_proj_v(s, t4)
                jb = 4 * s + t4 - 1
                if jb >= 0:
                    emit_sc(jb)
                    if jb >= 2:
                        emit_pv(jb - 2)
                        p_tiles.pop(jb - 3, None)
            if s == 0:
                emit_dma_x(3)
        emit_sc(NB - 1)
        for q in (NB - 3, NB - 2, NB - 1):
            emit_pv(q)

    nc.compile()
    return nc


def _host_inputs(x, Wqkv):
    """Per-core input maps: shard batch x head-group, pre-transpose, bf16."""
    import ml_dtypes

    bf16 = ml_dtypes.bfloat16
    scale = float(D) ** -0.5
    r = np.arange(128, dtype=np.float32)[:, None]
    ci = np.arange(128, dtype=np.float32)[None, :]
    # type 0: (c <= r)  applied to the seg of query block jb+1
    # type 1: (c >= r)  applied to the seg of query block jb-1
    msk = np.stack([(ci <= r), (ci >= r)], axis=1).astype(np.float32)  # [128,2,128]
    msk = np.repeat(msk[:, :, None, :], HPC, axis=2)  # [128, 2, HPC, 128]
    msk = np.ascontiguousarray(msk).astype(bf16)

    x = np.asarray(x, dtype=np.float32)
    Wqkv = np.asarray(Wqkv, dtype=np.float32)
    xT = [np.ascontiguousarray(x[b].T).astype(bf16) for b in range(B)]
    in_maps = []
    for core in range(N_CORES):
        b, hg = divmod(core, N_CORES // B)
        rows = slice(hg * HPC * D, (hg + 1) * HPC * D)
        wcat = np.concatenate(
            [
                Wqkv[0 * C:1 * C][rows] * scale,
                Wqkv[1 * C:2 * C][rows],
                Wqkv[2 * C:3 * C][rows],
            ],
            axis=0,
        )
        in_maps.append(
            {
                "xT": xT[b],
                "wT": np.ascontiguousarray(wcat.T).astype(bf16),
                "msk": msk,
            }
        )
    return in_maps


def _gather(results):
    out = np.empty((B, T, C), dtype=np.float32)
    for core in range(N_CORES):
        b, hg = divmod(core, N_CORES // B)
        out[b, :, hg * HPC * D:(hg + 1) * HPC * D] = results[core]["out"]
    return out


def kernel(x, Wqkv):
    from concourse.bass_utils import run_bass_kernel_spmd

    key = PDT_NAME
    if key not in _PROGRAM_CACHE:
        _PROGRAM_CACHE[key] = _build_program(key)
    nc = _PROGRAM_CACHE[key]
    in_maps = _host_inputs(x, Wqkv)
    res = run_bass_kernel_spmd(nc, in_maps, list(range(N_CORES)))
    return _gather(res.results)
